# revision 8
# baseline (speedup 1.0000x reference)
"""Cross-attention kernel for Trainium2, distributed over 8 NeuronCores.

Sharding: batch x head parallel. Cores 0-3 handle batch 0, cores 4-7 batch 1.
Within a team of 4, core r handles heads 4r..4r+3 (channel slice 256r..256r+256).

Per core:
  - KV projection for its 256 k-channels + 256 v-channels (tensor parallel,
    contraction over full D with host-pretransposed context/W_kv)
  - k LayerNorm: partial (sum, sumsq) per row + 16KB AllReduce within team
  - q LayerNorm: full-row stats from x[b], normalize only its channel slice
  - attention for its 4 heads, computed transposed (simT[j,i] = k.q) with
    softmax denominators from an appended ones-column in v (no max
    subtraction: |sim*scale| <= ~6 for this problem, exp stays in fp32 range)
  - attention output produced transposed [256, NQ]; AllGather within team
    -> [1024, NQ] per 512-column block; each core computes a disjoint
    256-OUTPUT-COLUMN slice of the output projection over all rows
    (no redundant work). Host assembles the 8 column slices.
"""

import numpy as np

import concourse.bass as bass
import concourse.mybir as mybir
import concourse.tile as tile
from concourse import bacc
from concourse.bass_utils import run_bass_kernel_spmd
from concourse.masks import make_identity

B, NQ, NK, D, H, DH = 2, 2048, 2048, 1024, 16, 64
NCORES = 8
TEAM = 4
HPC = 4            # heads per core
DSL = HPC * DH     # 256: per-core channel slice
EPS = 1e-6
SCALE = DH ** -0.5
GROUPS = [[0, 1, 2, 3], [4, 5, 6, 7]]
FP32 = mybir.dt.float32
FP32R = mybir.dt.float32r
NT = NQ // 128     # 16 row tiles
KC = D // 128      # 8 contraction chunks

_CACHE: dict = {}
MOCK_COLL = False  # replace collectives with local DMA (for TimelineSim)


def _bcast_ap(t, parts):
    ap = t.ap() if hasattr(t, "ap") and not isinstance(t, bass.AP) else t
    return bass.AP(tensor=ap.tensor, offset=ap.offset,
                   ap=[[0, parts]] + list(ap.ap))


def _build():
    nc = bacc.Bacc("TRN2", target_bir_lowering=False, debug=False,
                   num_devices=NCORES)
    x_b = nc.declare_dram_parameter("x_b", [NQ, D], FP32, isOutput=False)
    ctxT = nc.declare_dram_parameter("ctxT", [D, NK], FP32, isOutput=False)
    wkvT = nc.declare_dram_parameter("wkvT", [D, 2 * DSL], FP32, isOutput=False)
    woutT = nc.declare_dram_parameter("woutT", [D, DSL], FP32, isOutput=False)
    bout = nc.declare_dram_parameter("bout", [DSL], FP32, isOutput=False)
    gq_s = nc.declare_dram_parameter("gq_s", [DSL], FP32, isOutput=False)
    bq_s = nc.declare_dram_parameter("bq_s", [DSL], FP32, isOutput=False)
    gk_s = nc.declare_dram_parameter("gk_s", [DSL], FP32, isOutput=False)
    bk_s = nc.declare_dram_parameter("bk_s", [DSL], FP32, isOutput=False)
    y_full = nc.declare_dram_parameter("y_full", [NQ, DSL], FP32, isOutput=True)

    stats_dram = nc.dram_tensor("stats_dram", [128, 2 * NT], FP32)
    statsr_dram = nc.dram_tensor("statsr_dram", [128, 2 * NT], FP32)
    aoT_blk = [nc.dram_tensor(f"aoT_blk{i}", [DSL, 512], FP32) for i in range(4)]
    agT_blk = [nc.dram_tensor(f"agT_blk{i}", [D, 512], FP32) for i in range(4)]
    cs_dram = nc.dram_tensor("cs_dram", [HPC * 4, 512], FP32)

    ctxT_r = ctxT.ap().rearrange("(k p) m -> p k m", p=128)    # [128, 8, NK]
    wkvT_r = wkvT.ap().rearrange("(k p) n -> p k n", p=128)    # [128, 8, 512]
    woutT_r = woutT.ap().rearrange("(k p) n -> p k n", p=128)  # [128, 8, DSL]

    with tile.TileContext(nc) as tc:
        with (
            tc.tile_pool(name="singles", bufs=1) as singles,
            tc.tile_pool(name="ld", bufs=3) as ld,
            tc.tile_pool(name="work", bufs=3) as work,
            tc.tile_pool(name="psmm", bufs=2, space="PSUM") as psmm,
            tc.tile_pool(name="pssim", bufs=2, space="PSUM") as pssim,
            tc.tile_pool(name="psout", bufs=2, space="PSUM") as psout,
        ):
            # --- persistent sbuf ---
            wkv_sb = singles.tile([128, KC, 2 * DSL], FP32R)
            nc.sync.dma_start(out=wkv_sb, in_=wkvT_r.bitcast(FP32R))
            wout_sb = singles.tile([128, KC, DSL], FP32R)
            nc.sync.dma_start(out=wout_sb, in_=woutT_r.bitcast(FP32R))
            ident = singles.tile([128, 128], FP32)
            make_identity(nc, ident)
            eps_sb = singles.tile([128, 1], FP32)
            nc.vector.memset(eps_sb, EPS)
            def _col_ap(param, cb):
                ap = param.ap()
                return bass.AP(tensor=ap.tensor, offset=128 * cb,
                               ap=[[1, 128], [1, 1]])

            gqT = [singles.tile([128, 1], FP32, name=f"gqT{cb}") for cb in range(2)]
            bqT = [singles.tile([128, 1], FP32, name=f"bqT{cb}") for cb in range(2)]
            gkT = [singles.tile([128, 1], FP32, name=f"gkT{cb}") for cb in range(2)]
            bkT = [singles.tile([128, 1], FP32, name=f"bkT{cb}") for cb in range(2)]
            for cb in range(2):
                nc.sync.dma_start(out=gqT[cb], in_=_col_ap(gq_s, cb))
                nc.sync.dma_start(out=bqT[cb], in_=_col_ap(bq_s, cb))
                nc.sync.dma_start(out=gkT[cb], in_=_col_ap(gk_s, cb))
                nc.sync.dma_start(out=bkT[cb], in_=_col_ap(bk_s, cb))
            bout_b = singles.tile([128, DSL], FP32)
            nc.sync.dma_start(out=bout_b, in_=_bcast_ap(bout, 128))

            k_nat = singles.tile([128, NT, DSL], FP32)
            # v for 4 heads, interleaved [head][DH + ones + pad] per row tile
            vh_sb = singles.tile([128, NT, HPC * (DH + 2)], FP32R, name="vh")
            for h in range(HPC):
                o = h * (DH + 2)
                nc.vector.memset(
                    vh_sb[:, :, o + DH:o + DH + 1].bitcast(FP32), 1.0)
                nc.vector.memset(
                    vh_sb[:, :, o + DH + 1:o + DH + 2].bitcast(FP32), 0.0)
            qT_sb = [singles.tile([128, NT, 128], FP32R, tag=f"qT{cb}",
                                  name=f"qT{cb}") for cb in range(2)]
            kT_sb = [singles.tile([128, NT, 128], FP32R, tag=f"kT{cb}",
                                  name=f"kT{cb}") for cb in range(2)]
            aoT_sb = [singles.tile([128, NQ], FP32, tag=f"aoT{cb}",
                                   name=f"aoT{cb}") for cb in range(2)]
            stats_sb = singles.tile([128, NT, 2], FP32)
            statsr_sb = singles.tile([128, NT, 2], FP32)
            mean_all = singles.tile([128, NT], FP32)
            var_all = singles.tile([128, NT], FP32)
            rstd_all = singles.tile([128, NT], FP32)

            # --- stage A+C interleaved: kv-proj, k stats, q LN, q transpose ---
            for t in range(NT):
                # A: kv projection for NK row tile t
                ctx_sb = ld.tile([128, KC, 128], FP32R, tag="ctx")
                nc.sync.dma_start(out=ctx_sb,
                                  in_=ctxT_r[:, :, 128 * t:128 * (t + 1)]
                                  .bitcast(FP32R))
                kv_ps = psmm.tile([128, 2 * DSL], FP32, tag="mm512")
                for kk in range(KC):
                    nc.tensor.matmul(kv_ps, lhsT=ctx_sb[:, kk, :],
                                     rhs=wkv_sb[:, kk, :],
                                     start=(kk == 0), stop=(kk == KC - 1))
                nc.vector.tensor_copy(k_nat[:, t, :], kv_ps[:, 0:DSL])
                # v: one strided copy for all 4 heads
                v_dst = vh_sb[:, t, :].rearrange("p (h c) -> p h c", h=HPC)
                nc.vector.tensor_copy(
                    v_dst[:, :, 0:DH],
                    kv_ps[:, DSL:2 * DSL].rearrange("p (h c) -> p h c", h=HPC))
                # k partial stats (pre-norm)
                nc.vector.reduce_sum(out=stats_sb[:, t, 0:1],
                                     in_=k_nat[:, t, :],
                                     axis=mybir.AxisListType.X)
                sq_scr = work.tile([128, DSL], FP32, tag="sqscr", bufs=2)
                nc.vector.tensor_mul(sq_scr, k_nat[:, t, :], k_nat[:, t, :])
                nc.vector.reduce_sum(out=stats_sb[:, t, 1:2], in_=sq_scr,
                                     axis=mybir.AxisListType.X)

                # C: q LayerNorm for NQ row tile t
                x_sb = ld.tile([128, D], FP32, tag="x")
                nc.sync.dma_start(out=x_sb, in_=x_b[128 * t:128 * (t + 1), :])
                bn = work.tile([128, 2, 6], FP32, tag="bn")
                nc.vector.bn_stats(out=bn[:, 0, :], in_=x_sb[:, 0:512])
                nc.vector.bn_stats(out=bn[:, 1, :], in_=x_sb[:, 512:1024])
                mv = work.tile([128, 2], FP32, tag="mv")
                nc.vector.bn_aggr(out=mv, in_=bn)
                sdev = work.tile([128, 1], FP32, tag="sdev")
                nc.scalar.activation(sdev, mv[:, 1:2],
                                     mybir.ActivationFunctionType.Sqrt,
                                     bias=eps_sb)
                rq = work.tile([128, 1], FP32, tag="rq")
                nc.vector.reciprocal(rq, sdev)
                q_nat = work.tile([128, DSL], FP32, tag="qn")
                nc.vector.tensor_scalar(out=q_nat, in0=x_sb[:, 0:DSL],
                                        scalar1=mv[:, 0:1], scalar2=rq,
                                        op0=mybir.AluOpType.subtract,
                                        op1=mybir.AluOpType.mult)
                for cb in range(2):
                    tp_ps = pssim.tile([128, 128], FP32, tag="sim")
                    nc.tensor.transpose(tp_ps,
                                        q_nat[:, 128 * cb:128 * (cb + 1)],
                                        ident)
                    # fused copy+scale from PSUM
                    nc.vector.tensor_scalar(out=qT_sb[cb][:, t, :],
                                            in0=tp_ps,
                                            scalar1=gqT[cb], scalar2=bqT[cb],
                                            op0=mybir.AluOpType.mult,
                                            op1=mybir.AluOpType.add)

            # --- stage B: AllReduce k stats within team ---
            nc.sync.dma_start(out=stats_dram[:, :],
                              in_=stats_sb.rearrange("p t s -> p (t s)"))
            if MOCK_COLL:
                nc.sync.dma_start(out=statsr_dram[:, :], in_=stats_dram[:, :])
            else:
                nc.gpsimd.collective_compute(
                    "AllReduce", mybir.AluOpType.add, replica_groups=GROUPS,
                    ins=[stats_dram.ap().opt()], outs=[statsr_dram.ap().opt()])
            nc.sync.dma_start(out=statsr_sb.rearrange("p t s -> p (t s)"),
                              in_=statsr_dram[:, :])
            nc.vector.tensor_scalar_mul(mean_all, in0=statsr_sb[:, :, 0],
                                        scalar1=1.0 / D)
            nc.vector.tensor_scalar_mul(var_all, in0=statsr_sb[:, :, 1],
                                        scalar1=1.0 / D)
            m2 = work.tile([128, NT], FP32, tag="m2")
            nc.vector.tensor_mul(m2, mean_all, mean_all)
            nc.vector.tensor_sub(var_all, var_all, m2)
            nc.scalar.activation(var_all, var_all,
                                 mybir.ActivationFunctionType.Sqrt, bias=eps_sb)
            nc.vector.reciprocal(rstd_all, var_all)

            # --- stage D+E: k LN apply + k transpose ---
            for t in range(NT):
                nc.vector.tensor_scalar(out=k_nat[:, t, :], in0=k_nat[:, t, :],
                                        scalar1=mean_all[:, t:t + 1],
                                        scalar2=rstd_all[:, t:t + 1],
                                        op0=mybir.AluOpType.subtract,
                                        op1=mybir.AluOpType.mult)
                for cb in range(2):
                    tp_ps = pssim.tile([128, 128], FP32, tag="sim")
                    nc.tensor.transpose(tp_ps,
                                        k_nat[:, t, 128 * cb:128 * (cb + 1)],
                                        ident)
                    nc.vector.tensor_scalar(out=kT_sb[cb][:, t, :],
                                            in0=tp_ps,
                                            scalar1=gkT[cb], scalar2=bkT[cb],
                                            op0=mybir.AluOpType.mult,
                                            op1=mybir.AluOpType.add)

            # --- stage F+G+H fused: attention -> per-block AllGather ->
            # out-projection (256 output columns), pipelined over 512-column
            # blocks of NQ ---
            for iblk in range(4):
                for h in range(HPC):
                    cb, hh = h // 2, h % 2
                    khT = kT_sb[cb][64 * hh:64 * (hh + 1), :, :]
                    qhT = qT_sb[cb][64 * hh:64 * (hh + 1), :, :]
                    vo = h * (DH + 2)
                    oT_ps = psout.tile([DH + 2, 512], FP32, tag="oT")
                    for jp in range(NT // 2):
                        s_ps = pssim.tile([128, 2, 512], FP32, tag="sim")
                        for jj in range(2):
                            j = 2 * jp + jj
                            nc.tensor.matmul(
                                s_ps[:, jj, :], lhsT=khT[:, j, :],
                                rhs=qhT[:, 4 * iblk:4 * (iblk + 1), :],
                                start=True, stop=True)
                        e_sb = work.tile([128, 2, 512], FP32R, tag="exp",
                                         bufs=4)
                        nc.scalar.activation(
                            e_sb.rearrange("p a b -> p (a b)"),
                            s_ps.rearrange("p a b -> p (a b)"),
                            mybir.ActivationFunctionType.Exp, scale=SCALE)
                        for jj in range(2):
                            j = 2 * jp + jj
                            nc.tensor.matmul(oT_ps,
                                             lhsT=vh_sb[:, j, vo:vo + DH + 2],
                                             rhs=e_sb[:, jj, :],
                                             start=(j == 0), stop=(j == NT - 1))
                    # normalize: row DH of oT_ps holds the softmax
                    # denominators; invert on DVE, then DMA-broadcast
                    csi = 4 * h + iblk
                    cs = work.tile([1, 512], FP32, tag="cs", bufs=2)
                    nc.vector.reciprocal(cs, oT_ps[DH:DH + 1, :])
                    nc.sync.dma_start(out=cs_dram[csi:csi + 1, :], in_=cs)
                    csb = work.tile([64, 512], FP32, tag="csb", bufs=2)
                    nc.sync.dma_start(out=csb,
                                      in_=_bcast_ap(cs_dram[csi, :], 64))
                    nc.vector.tensor_mul(
                        aoT_sb[cb][64 * hh:64 * (hh + 1),
                                   512 * iblk:512 * (iblk + 1)],
                        oT_ps[0:DH, :], csb)
                # gather this column block and project it while later
                # blocks are still in flight
                for cb in range(2):
                    nc.sync.dma_start(
                        out=aoT_blk[iblk][128 * cb:128 * (cb + 1), :],
                        in_=aoT_sb[cb][:, 512 * iblk:512 * (iblk + 1)])
                if MOCK_COLL:
                    nc.sync.dma_start(out=agT_blk[iblk][0:DSL, :],
                                      in_=aoT_blk[iblk][:, :])
                else:
                    nc.gpsimd.collective_compute(
                        "AllGather", mybir.AluOpType.bypass,
                        replica_groups=GROUPS,
                        ins=[aoT_blk[iblk].ap().opt()],
                        outs=[agT_blk[iblk].ap().opt()])
                ag_r = agT_blk[iblk].ap().rearrange("(k p) n -> p k n", p=128)
                for sub in range(4):
                    nt = 4 * iblk + sub
                    ag_sb = ld.tile([128, KC, 128], FP32R, tag="ctx",
                                    name="ag_sb")
                    nc.sync.dma_start(
                        out=ag_sb,
                        in_=ag_r[:, :, 128 * sub:128 * (sub + 1)].bitcast(FP32R))
                    y_ps = psmm.tile([128, DSL], FP32, tag="mm512",
                                     name="y_ps")
                    for kk in range(KC):
                        nc.tensor.matmul(y_ps, lhsT=ag_sb[:, kk, :],
                                         rhs=wout_sb[:, kk, :],
                                         start=(kk == 0), stop=(kk == KC - 1))
                    y_sb = work.tile([128, DSL], FP32, tag="y", bufs=2)
                    nc.vector.tensor_add(y_sb, y_ps, bout_b)
                    nc.sync.dma_start(out=y_full[128 * nt:128 * (nt + 1), :],
                                      in_=y_sb)

    nc.finalize()
    return nc


def kernel(x, context, gq, bq, gk, bk, W_kv, W_out, b_out):
    x = np.asarray(x, dtype=np.float32)
    context = np.asarray(context, dtype=np.float32)
    gq = np.asarray(gq, dtype=np.float32)
    bq = np.asarray(bq, dtype=np.float32)
    gk = np.asarray(gk, dtype=np.float32)
    bk = np.asarray(bk, dtype=np.float32)
    W_kv = np.asarray(W_kv, dtype=np.float32)
    W_out = np.asarray(W_out, dtype=np.float32)
    b_out = np.asarray(b_out, dtype=np.float32)

    if "nc" not in _CACHE:
        _CACHE["nc"] = _build()
    nc = _CACHE["nc"]

    Wk, Wv = W_kv[:D], W_kv[D:]
    in_maps = []
    for c in range(NCORES):
        b, r = c // TEAM, c % TEAM
        sl = slice(DSL * r, DSL * (r + 1))
        wkvT_c = np.ascontiguousarray(
            np.concatenate([Wk[sl], Wv[sl]], axis=0).T)
        in_maps.append({
            # roll channels so this core's q slice sits at cols 0:DSL
            # (LayerNorm full-row stats are permutation invariant)
            "x_b": np.ascontiguousarray(np.roll(x[b], -DSL * r, axis=1)),
            "ctxT": np.ascontiguousarray(context[b].T),
            "wkvT": wkvT_c,
            "woutT": np.ascontiguousarray(W_out.T[:, sl]),
            "bout": np.ascontiguousarray(b_out[sl]),
            "gq_s": np.ascontiguousarray(gq[sl]),
            "bq_s": np.ascontiguousarray(bq[sl]),
            "gk_s": np.ascontiguousarray(gk[sl]),
            "bk_s": np.ascontiguousarray(bk[sl]),
        })

    _CACHE["in_maps"] = in_maps
    try:
        res = run_bass_kernel_spmd(nc, in_maps, list(range(NCORES))).results
    except Exception:
        # transient runtime failures (device wedged from a prior run) --
        # one retry typically succeeds
        res = run_bass_kernel_spmd(nc, in_maps, list(range(NCORES))).results
    y = np.empty((B, NQ, D), dtype=np.float32)
    for c in range(NCORES):
        b, r = c // TEAM, c % TEAM
        y[b, :, DSL * r:DSL * (r + 1)] = res[c]["y_full"]
    return y
